# revision 2
# baseline (speedup 1.0000x reference)
"""Trainium2 Bass kernel for nn_CompProbModel_76948634075343.

Reference semantics: a completion-probability model that builds a
[B=8, N=6600, T=40, J=22] interception-probability tensor and then collapses
it with three gathers: time-of-flight bin -> targeted receiver -> ball
landing field cell, yielding one scalar per play.

Key algebraic observation: the gathers commute with everything upstream, so
per play we only need the physics at ONE field location (the ball landing
cell) and ONE time bin (the time of flight).  That reduces the computation
to a [22]-player vector pipeline per play:

    d      = ball_cell_xy - player_xy                      # [22, 2]
    s0     = clip(<d, v> / |d|, +-S_MAX)
    t_lt   = (S_MAX - s0)/A_MAX      (time to reach top speed)
    d_lt   = t_lt (s0 + S_MAX)/2     (distance covered by then)
    t_lt2  = -s0/A + sqrt((s0/A)^2 + 2|d|/A)
    t_ltf  = min(t_lt, t_lt2)        == where(d_lt > |d|, t_lt2, t_lt)
    t_tot  = t_ltf + max(|d| - d_lt, 0)/S_MAX
    p_j    = sigmoid(k (T_tof - t_tot))
    P_def  = prod_j (1 - p_j (1 - team_j))
    out    = p_recv * P_def * team_recv + 0.001

(The where() -> min() rewrite is exact: t_lt2 is the accelerating-phase
arrival time, which is smaller than t_lt exactly when the target is closer
than the speed-saturation distance d_lt; the branches agree at the
boundary.  Likewise clip(d_lt, 0, |d|) -> max(|d|-d_lt, 0) because
d_lt >= 0 always: t_lt >= 0 and s0+S_MAX >= 0 after the clip.)

Sharding: pure data parallel over the batch (play) dimension, one play per
NeuronCore (8 plays, 8 cores).  Each core receives its play's frame
(flattened, concatenated with a 22-entry constant) in a single 1.3KB DMA,
computes the scalar fully on-device, and the host concatenates the 8
scalars.

On-device layout: everything lives in ONE SBUF partition as [1, n] row
vectors (players along the free dim), so all reductions (max over receiver
weights, pairwise x/y reductions, product over defenders via
tensor_tensor_scan, receiver pick via scalar_tensor_tensor's fused
accumulator) are native free-dim DVE ops.  The data-dependent gathers
become arithmetic: field x = 0.5 + ball_end_x, field y = 0.5 + ball_end_y,
T = 0.1 * round(tof), receiver via max + is_equal one-hot.

ACT usage is grouped by table set to minimize the ~1.3us ACT table loads:
the sqrt set is pre-loaded at kernel start (a warm activation issued
concurrently with the input DMA), both Sqrt calls run from it, and the
single switch to the sigmoid set overlaps the DVE work between the second
Sqrt and the Sigmoid.  Division is done with the DVE's iterative-divide
reciprocal (accurate), and the sqrt/sigmoid ACT tables were measured on
this hardware at ~2e-7 max relative error.
"""

import numpy as np

B, J, F = 8, 22, 14
NX, NY, NT = 120, 55, 40
A_MAX = 7.25
S_MAX = 9.25
K_SIG = float(np.float32(3.14 / (1.732 * 0.5)))

_IN_LEN = J * F + J  # frame flat (308) ++ receiver argmax weights (22)


def _build_program():
    import concourse.bacc as bacc
    import concourse.tile as tile
    from concourse import mybir
    from concourse.vector_clock import ScopedClock

    class LeanTileContext(tile.TileContext):
        """TileContext with a trimmed end-of-kernel tail.

        The stock tail is drain + all-engine barrier + semaphore clear +
        all-engine barrier.  The clear must stay (the loaded NEFF is
        re-executed across invocations and semaphores must return to zero
        - verified empirically: without it a second kernel() call hangs),
        and the first barrier must stay (no engine may clear a semaphore
        another engine still waits on).  The SECOND barrier is droppable:
        after barrier one only the clearing engine touches semaphores, and
        the runtime already waits for every engine's retirement, so the
        clear is guaranteed to complete before the next execution.
        """

        def _drain_and_barrier(self, tick_clock, wait_clock):
            drain_inst = self.nc.sync.drain()
            wait_clock.add_sem_waits(
                drain_inst.ins, ScopedClock({None: tick_clock.global_clock})
            )
            self.nc.all_engine_barrier()
            popped = self.nc._tile_sem_poison_stack.pop()
            assert popped is self._sem_poison
            self.nc.clear_and_free_semaphores(list(self.sems.allocated().values()))

    fp32 = mybir.dt.float32
    Alu = mybir.AluOpType
    Act = mybir.ActivationFunctionType
    X = mybir.AxisListType.X

    nc = bacc.Bacc("TRN2", target_bir_lowering=False, debug=False, num_devices=B)
    in_d = nc.dram_tensor("inp", [1, _IN_LEN], fp32, kind="ExternalInput")
    out_d = nc.dram_tensor("out", [1, 1], fp32, kind="ExternalOutput")

    with LeanTileContext(nc) as tc:
        with tc.tile_pool(name="p", bufs=1) as pool:
            v = nc.vector
            sc = nc.scalar

            def tl(tag, n=J):
                return pool.tile([1, n], fp32, tag=tag, name=tag)

            # ---- load + ACT sqrt-set warm (concurrent) ----------------
            inp = tl("inp", _IN_LEN)
            nc.sync.dma_start(inp[:], in_d[:], single_packet=True)
            warm = tl("warm", 1)
            nc.gpsimd.memset(warm[:], 0.0)
            sc.activation(warm[:], warm[:], Act.Sqrt)

            frj = inp[:, 0:J * F].rearrange("p (j f) -> p j f", f=F)
            pxy = frj[:, :, 1:3]   # [1,22,2] player (x, y)
            vxy = frj[:, :, 3:5]   # [1,22,2] player (vx, vy)
            team = frj[:, :, 7]
            rec = frj[:, :, 10]
            bx0 = inp[:, 11:12]
            by0 = inp[:, 12:13]
            tof0 = inp[:, 13:14]
            wdesc = inp[:, J * F:J * F + J]

            # ---- per-play prep ----------------------------------------
            # ball cell coords (x = 0.5 + bx, y = -0.5 + (by+1))
            star2 = tl("star2", 2)
            v.tensor_scalar(star2[:, 0:1], bx0, 0.5, None, Alu.add)
            v.tensor_scalar(star2[:, 1:2], by0, 0.5, None, Alu.add)
            # sigmoid bias k*T = (tof * 0.1) * k
            kt = tl("kt", 1)
            v.tensor_scalar(kt[:], tof0, 0.1, K_SIG, Alu.mult, Alu.mult)
            # receiver one-hot * team, defender weight
            rw = tl("rw")
            v.tensor_tensor(rw[:], rec, wdesc, Alu.mult)
            rmax = tl("rmax", 1)
            v.reduce_max(rmax[:], rw[:], axis=X)
            rmask = tl("rmask")
            v.tensor_scalar(rmask[:], rw[:], rmax[:], None, Alu.is_equal)
            rteam = tl("rteam")
            v.tensor_tensor(rteam[:], rmask[:], team, Alu.mult)
            wdef = tl("wdef")
            v.tensor_scalar(wdef[:], team, -1.0, 1.0, Alu.mult, Alu.add)

            # ---- time-to-intercept physics ----------------------------
            nd = tl("nd", 2 * J)  # interleaved (px-x*, py-y*) pairs = -d
            ndp = nd[:].rearrange("p (j c) -> p j c", c=2)
            v.tensor_scalar(ndp[:, :, 0], frj[:, :, 1], star2[:, 0:1], None,
                            Alu.subtract)
            v.tensor_scalar(ndp[:, :, 1], frj[:, :, 2], star2[:, 1:2], None,
                            Alu.subtract)
            sq = tl("sq", 2 * J)
            v.tensor_tensor(sq[:], nd[:], nd[:], Alu.mult)
            d2 = tl("d2")
            v.reduce_sum(d2[:], sq[:].rearrange("p (j c) -> p j c", c=2), axis=X)
            dv = tl("dv", 2 * J)
            v.tensor_tensor(dv[:].rearrange("p (j c) -> p j c", c=2), ndp, vxy,
                            Alu.mult)
            dotn = tl("dotn")  # = -<d, v>
            v.reduce_sum(dotn[:], dv[:].rearrange("p (j c) -> p j c", c=2), axis=X)

            dmag = tl("dmag")
            sc.activation(dmag[:], d2[:], Act.Sqrt)
            invd = tl("invd")
            v.reciprocal(invd[:], dmag[:])

            # m0 = clip(dotn/|d|, +-S) = -s0 ; w = m0/A
            m0 = tl("m0")
            v.tensor_tensor(m0[:], dotn[:], invd[:], Alu.mult)
            m0c = tl("m0c")
            v.tensor_scalar(m0c[:], m0[:], S_MAX, -S_MAX, Alu.min, Alu.max)
            w = tl("w")
            v.tensor_scalar(w[:], m0c[:], 1.0 / A_MAX, None, Alu.mult)
            t_lt = tl("t_lt")
            v.tensor_scalar(t_lt[:], w[:], S_MAX / A_MAX, None, Alu.add)
            u = tl("u")  # (S - m0)/2
            v.tensor_scalar(u[:], w[:], -A_MAX / 2.0, S_MAX / 2.0, Alu.mult, Alu.add)
            d_lt = tl("d_lt")
            v.tensor_tensor(d_lt[:], t_lt[:], u[:], Alu.mult)
            w2 = tl("w2")
            v.tensor_tensor(w2[:], w[:], w[:], Alu.mult)
            q = tl("q")
            v.scalar_tensor_tensor(q[:], dmag[:], 2.0 / A_MAX, w2[:], Alu.mult,
                                   Alu.add)
            r = tl("r")
            sc.activation(r[:], q[:], Act.Sqrt)
            t_lt2 = tl("t_lt2")
            v.tensor_tensor(t_lt2[:], w[:], r[:], Alu.add)
            t_ltf = tl("t_ltf")
            v.tensor_tensor(t_ltf[:], t_lt[:], t_lt2[:], Alu.min)

            dd = tl("dd")
            v.tensor_tensor(dd[:], dmag[:], d_lt[:], Alu.subtract)
            ddr = tl("ddr")
            v.tensor_scalar(ddr[:], dd[:], 0.0, None, Alu.max)
            t_tot = tl("t_tot")
            v.scalar_tensor_tensor(t_tot[:], ddr[:], 1.0 / S_MAX, t_ltf[:],
                                   Alu.mult, Alu.add)

            # p = sigmoid(-k t_tot + k T)
            p = tl("p")
            sc.activation(p[:], t_tot[:], Act.Sigmoid, scale=-K_SIG, bias=kt[:])

            # defender no-intercept product; receiver pick; final scale
            pw = tl("pw")
            v.tensor_tensor(pw[:], p[:], wdef[:], Alu.mult)
            dterm = tl("dterm")
            v.tensor_scalar(dterm[:], pw[:], -1.0, 1.0, Alu.mult, Alu.add)
            scan = tl("scan")
            v.tensor_tensor_scan(scan[:], dterm[:], dterm[:], 1.0, Alu.mult,
                                 Alu.bypass)
            j22 = tl("j22")
            s = tl("s", 1)
            v.scalar_tensor_tensor(j22[:], p[:], 0.0, rteam[:], Alu.bypass,
                                   Alu.mult, accum_out=s[:])
            res = tl("res", 1)
            v.tensor_scalar(res[:], s[:], scan[:, J - 1:J], 0.001, Alu.mult,
                            Alu.add)

            nc.sync.dma_start(out_d[:], res[:], single_packet=True)

    nc.compile()
    return nc


_CACHE = {}

CORE_IDS = list(range(B))


def _get_program():
    if "nc" not in _CACHE:
        _CACHE["nc"] = _build_program()
    return _CACHE["nc"]


def _unshard(res, frame: np.ndarray) -> np.ndarray:
    return np.array(
        [res.results[b]["out"][0, 0] for b in range(B)], dtype=np.float32
    )


def _in_maps(frame: np.ndarray):
    wdesc = np.arange(J, 0, -1, dtype=np.float32)
    return [
        {"inp": np.concatenate([frame[b].ravel(), wdesc]).reshape(1, _IN_LEN)}
        for b in range(B)
    ]


def kernel(frame: np.ndarray) -> np.ndarray:
    from concourse.bass_utils import run_bass_kernel_spmd

    frame = np.ascontiguousarray(frame, dtype=np.float32)
    assert frame.shape == (B, J, F), frame.shape

    nc = _get_program()
    # shard: play b -> core b
    out = run_bass_kernel_spmd(nc, _in_maps(frame), core_ids=list(range(B)))
    # unshard: concatenate the per-core scalars
    return np.array(
        [out.results[b]["out"][0, 0] for b in range(B)], dtype=np.float32
    )



# revision 4
# speedup vs baseline: 1.5196x; 1.5196x over previous
"""Trainium2 Bass kernel for nn_CompProbModel_76948634075343.

Reference semantics: a completion-probability model that builds a
[B=8, N=6600, T=40, J=22] interception-probability tensor and then collapses
it with three gathers: time-of-flight bin -> targeted receiver -> ball
landing field cell, yielding one scalar per play.

Key algebraic observation: the gathers commute with everything upstream, so
per play we only need the physics at ONE field location (the ball landing
cell) and ONE time bin (the time of flight).  That reduces the computation
to a [22]-player vector pipeline per play:

    d      = ball_cell_xy - player_xy                      # [22, 2]
    s0     = clip(<d, v> / |d|, +-S_MAX)
    t_lt   = (S_MAX - s0)/A_MAX      (time to reach top speed)
    d_lt   = t_lt (s0 + S_MAX)/2     (distance covered by then)
    t_lt2  = -s0/A + sqrt((s0/A)^2 + 2|d|/A)
    t_ltf  = min(t_lt, t_lt2)        == where(d_lt > |d|, t_lt2, t_lt)
    t_tot  = t_ltf + max(|d| - d_lt, 0)/S_MAX
    p_j    = sigmoid(k (T_tof - t_tot))
    P_def  = prod_j (1 - p_j (1 - team_j))
    out    = p_recv * P_def * team_recv + 0.001

(The where() -> min() rewrite is exact: t_lt2 is the accelerating-phase
arrival time, which is smaller than t_lt exactly when the target is closer
than the speed-saturation distance d_lt; the branches agree at the
boundary.  Likewise clip(d_lt, 0, |d|) -> max(|d|-d_lt, 0) because
d_lt >= 0 always: t_lt >= 0 and s0+S_MAX >= 0 after the clip.)

Sharding: pure data parallel over the batch (play) dimension, one play per
NeuronCore (8 plays, 8 cores).  Each core receives its play's frame
(flattened, concatenated with a 22-entry constant) in a single 1.3KB DMA,
computes the scalar fully on-device, and the host concatenates the 8
scalars.

On-device layout: everything lives in ONE SBUF partition as [1, n] row
vectors (players along the free dim), so all reductions (max over receiver
weights, pairwise x/y reductions, product over defenders via
tensor_tensor_scan, receiver pick via scalar_tensor_tensor's fused
accumulator) are native free-dim DVE ops.  The data-dependent gathers
become arithmetic: field x = 0.5 + ball_end_x, field y = 0.5 + ball_end_y,
T = 0.1 * round(tof), receiver via max + is_equal one-hot.

ACT usage is grouped by table set to minimize the ~1.3us ACT table loads:
the sqrt set is pre-loaded at kernel start (a warm activation issued
concurrently with the input DMA), both Sqrt calls run from it, and the
single switch to the sigmoid set overlaps the DVE work between the second
Sqrt and the Sigmoid.  Division is done with the DVE's iterative-divide
reciprocal (accurate), and the sqrt/sigmoid ACT tables were measured on
this hardware at ~2e-7 max relative error.
"""

import numpy as np

B, J, F = 8, 22, 14
NX, NY, NT = 120, 55, 40
A_MAX = 7.25
S_MAX = 9.25
K_SIG = float(np.float32(3.14 / (1.732 * 0.5)))

_IN_LEN = J * F + J  # frame flat (308) ++ receiver argmax weights (22)


def _build_program():
    import concourse.bacc as bacc
    import concourse.tile as tile
    from concourse import mybir
    from concourse.vector_clock import ScopedClock

    class LeanTileContext(tile.TileContext):
        """TileContext with a trimmed end-of-kernel tail.

        The stock tail is drain + all-engine barrier + semaphore clear +
        all-engine barrier.  The clear must stay (the loaded NEFF is
        re-executed across invocations and semaphores must return to zero
        - verified empirically: without it a second kernel() call hangs),
        and the first barrier must stay (no engine may clear a semaphore
        another engine still waits on).  The SECOND barrier is droppable:
        after barrier one only the clearing engine touches semaphores, and
        the runtime already waits for every engine's retirement, so the
        clear is guaranteed to complete before the next execution.
        """

        def _drain_and_barrier(self, tick_clock, wait_clock):
            drain_inst = self.nc.sync.drain()
            wait_clock.add_sem_waits(
                drain_inst.ins, ScopedClock({None: tick_clock.global_clock})
            )
            self.nc.all_engine_barrier()
            popped = self.nc._tile_sem_poison_stack.pop()
            assert popped is self._sem_poison
            self.nc.clear_and_free_semaphores(list(self.sems.allocated().values()))

    fp32 = mybir.dt.float32
    Alu = mybir.AluOpType
    Act = mybir.ActivationFunctionType
    X = mybir.AxisListType.X

    nc = bacc.Bacc("TRN2", target_bir_lowering=False, debug=False, num_devices=B)
    in_d = nc.dram_tensor("inp", [1, _IN_LEN], fp32, kind="ExternalInput")
    out_d = nc.dram_tensor("out", [1, 1], fp32, kind="ExternalOutput")

    with LeanTileContext(nc) as tc:
        with tc.tile_pool(name="p", bufs=1) as pool:
            v = nc.vector
            sc = nc.scalar

            def tl(tag, n=J):
                return pool.tile([1, n], fp32, tag=tag, name=tag)

            # ---- load + ACT sqrt-set warm (concurrent) ----------------
            inp = tl("inp", _IN_LEN)
            nc.sync.dma_start(inp[:], in_d[:], single_packet=True)
            warm = tl("warm", 1)
            nc.gpsimd.memset(warm[:], 0.0)
            sc.activation(warm[:], warm[:], Act.Sqrt)

            frj = inp[:, 0:J * F].rearrange("p (j f) -> p j f", f=F)
            pxy = frj[:, :, 1:3]   # [1,22,2] player (x, y)
            vxy = frj[:, :, 3:5]   # [1,22,2] player (vx, vy)
            team = frj[:, :, 7]
            rec = frj[:, :, 10]
            bx0 = inp[:, 11:12]
            by0 = inp[:, 12:13]
            tof0 = inp[:, 13:14]
            wdesc = inp[:, J * F:J * F + J]

            # ---- per-play prep ----------------------------------------
            # ball cell coords (x = 0.5 + bx, y = -0.5 + (by+1))
            star2 = tl("star2", 2)
            v.tensor_scalar(star2[:, 0:1], bx0, 0.5, None, Alu.add)
            v.tensor_scalar(star2[:, 1:2], by0, 0.5, None, Alu.add)
            # sigmoid bias k*T = (tof * 0.1) * k
            kt = tl("kt", 1)
            v.tensor_scalar(kt[:], tof0, 0.1, K_SIG, Alu.mult, Alu.mult)
            # receiver one-hot * team, defender weight
            rw = tl("rw")
            v.tensor_tensor(rw[:], rec, wdesc, Alu.mult)
            rmax = tl("rmax", 1)
            v.reduce_max(rmax[:], rw[:], axis=X)
            rmask = tl("rmask")
            v.tensor_scalar(rmask[:], rw[:], rmax[:], None, Alu.is_equal)
            rteam = tl("rteam")
            v.tensor_tensor(rteam[:], rmask[:], team, Alu.mult)
            wdef = tl("wdef")
            v.tensor_scalar(wdef[:], team, -1.0, 1.0, Alu.mult, Alu.add)

            # ---- time-to-intercept physics ----------------------------
            nd = tl("nd", 2 * J)  # interleaved (px-x*, py-y*) pairs = -d
            ndp = nd[:].rearrange("p (j c) -> p j c", c=2)
            v.tensor_scalar(ndp[:, :, 0], frj[:, :, 1], star2[:, 0:1], None,
                            Alu.subtract)
            v.tensor_scalar(ndp[:, :, 1], frj[:, :, 2], star2[:, 1:2], None,
                            Alu.subtract)
            sq = tl("sq", 2 * J)
            v.tensor_tensor(sq[:], nd[:], nd[:], Alu.mult)
            d2 = tl("d2")
            v.reduce_sum(d2[:], sq[:].rearrange("p (j c) -> p j c", c=2), axis=X)
            dv = tl("dv", 2 * J)
            v.tensor_tensor(dv[:].rearrange("p (j c) -> p j c", c=2), ndp, vxy,
                            Alu.mult)
            dotn = tl("dotn")  # = -<d, v>
            v.reduce_sum(dotn[:], dv[:].rearrange("p (j c) -> p j c", c=2), axis=X)

            dmag = tl("dmag")
            sc.activation(dmag[:], d2[:], Act.Sqrt)
            invd = tl("invd")
            v.reciprocal(invd[:], dmag[:])

            # m0 = clip(dotn/|d|, +-S) = -s0 ; w = m0/A
            m0 = tl("m0")
            v.tensor_tensor(m0[:], dotn[:], invd[:], Alu.mult)
            m0c = tl("m0c")
            v.tensor_scalar(m0c[:], m0[:], S_MAX, -S_MAX, Alu.min, Alu.max)
            w = tl("w")
            v.tensor_scalar(w[:], m0c[:], 1.0 / A_MAX, None, Alu.mult)
            t_lt = tl("t_lt")
            v.tensor_scalar(t_lt[:], w[:], S_MAX / A_MAX, None, Alu.add)
            u = tl("u")  # (S - m0)/2
            v.tensor_scalar(u[:], w[:], -A_MAX / 2.0, S_MAX / 2.0, Alu.mult, Alu.add)
            d_lt = tl("d_lt")
            v.tensor_tensor(d_lt[:], t_lt[:], u[:], Alu.mult)
            w2 = tl("w2")
            v.tensor_tensor(w2[:], w[:], w[:], Alu.mult)
            q = tl("q")
            v.scalar_tensor_tensor(q[:], dmag[:], 2.0 / A_MAX, w2[:], Alu.mult,
                                   Alu.add)
            r = tl("r")
            sc.activation(r[:], q[:], Act.Sqrt)
            t_lt2 = tl("t_lt2")
            v.tensor_tensor(t_lt2[:], w[:], r[:], Alu.add)
            t_ltf = tl("t_ltf")
            v.tensor_tensor(t_ltf[:], t_lt[:], t_lt2[:], Alu.min)

            dd = tl("dd")
            v.tensor_tensor(dd[:], dmag[:], d_lt[:], Alu.subtract)
            ddr = tl("ddr")
            v.tensor_scalar(ddr[:], dd[:], 0.0, None, Alu.max)
            t_tot = tl("t_tot")
            v.scalar_tensor_tensor(t_tot[:], ddr[:], 1.0 / S_MAX, t_ltf[:],
                                   Alu.mult, Alu.add)

            # p = sigmoid(-k t_tot + k T)
            p = tl("p")
            sc.activation(p[:], t_tot[:], Act.Sigmoid, scale=-K_SIG, bias=kt[:])

            # defender no-intercept product; receiver pick; final scale
            pw = tl("pw")
            v.tensor_tensor(pw[:], p[:], wdef[:], Alu.mult)
            dterm = tl("dterm")
            v.tensor_scalar(dterm[:], pw[:], -1.0, 1.0, Alu.mult, Alu.add)
            scan = tl("scan")
            v.tensor_tensor_scan(scan[:], dterm[:], dterm[:], 1.0, Alu.mult,
                                 Alu.bypass)
            j22 = tl("j22")
            s = tl("s", 1)
            v.scalar_tensor_tensor(j22[:], p[:], 0.0, rteam[:], Alu.bypass,
                                   Alu.mult, accum_out=s[:])
            res = tl("res", 1)
            v.tensor_scalar(res[:], s[:], scan[:, J - 1:J], 0.001, Alu.mult,
                            Alu.add)

            nc.sync.dma_start(out_d[:], res[:], single_packet=True)

    nc.compile()
    return nc


_CACHE = {}

CORE_IDS = list(range(B))


def _get_program():
    if "nc" not in _CACHE:
        _CACHE["nc"] = _build_program()
    return _CACHE["nc"]


def _unshard(res, frame: np.ndarray) -> np.ndarray:
    return np.array(
        [res.results[b]["out"][0, 0] for b in range(B)], dtype=np.float32
    )


def _in_maps(frame: np.ndarray):
    wdesc = np.arange(J, 0, -1, dtype=np.float32)
    return [
        {"inp": np.concatenate([frame[b].ravel(), wdesc]).reshape(1, _IN_LEN)}
        for b in range(B)
    ]


def kernel(frame: np.ndarray) -> np.ndarray:
    from concourse.bass_utils import run_bass_kernel_spmd

    frame = np.ascontiguousarray(frame, dtype=np.float32)
    assert frame.shape == (B, J, F), frame.shape

    nc = _get_program()
    # shard: play b -> core b
    out = run_bass_kernel_spmd(nc, _in_maps(frame), core_ids=list(range(B)))
    # unshard: concatenate the per-core scalars
    return np.array(
        [out.results[b]["out"][0, 0] for b in range(B)], dtype=np.float32
    )

